# revision 25
# baseline (speedup 1.0000x reference)
"""ECT transform kernel for Trainium2, SPMD over 8 NeuronCores.

Math (per sample b):
    nh[b,n,t] = sum_d x[b,n,d] * v[d,t]
    ect[b,r,t] = sum_n sigmoid(SCALE*(lin[r] - nh[b,n,t]))
    out[b] = ect[b] / max_{r,t} ect[b]

Sharding: data-parallel over batch (B=16 -> 2 samples per core).

Per-core program (partitions p = (b, t), b in {0,1}, t in 0..63):
  - PE: nh[(b,t), n] = w18.T @ x18, K=18 = (split, d, b-indicator) with
    bf16-split precision (x_hi*v_hi + x_lo*v_hi + x_hi*v_lo), 4 matmuls of
    N=512 into one PSUM tile (128, 2048).  Single-pass bf16 (fp32 matmul
    runs as two passes on this PE).
  - ACT: for each r: sigmoid(-SCALE*nh + SCALE*lin[r]) over the whole PSUM
    tile, with the fused per-partition accumulator (accum_out) producing
    ect[(b,t), r] directly.  The r axis is never materialized.
  - normalize: free-dim max (DVE) + per-half partition max (GPSIMD at
    base partition 0), reciprocal, per-partition scale.
  - PE-transpose (identity matmul) to (r, (b,t)) and two contiguous
    output DMAs (strided 4-byte DMA to DRAM measured ~24us; transpose
    path is ~1us).
"""

import numpy as np
import ml_dtypes

import concourse.bacc as bacc
import concourse.tile as tile
from concourse import mybir
from concourse.bass_utils import run_bass_kernel_spmd
from concourse.masks import make_identity

B = 16
N = 2048
D = 3
T = 64
R = 64
RADIUS = 1.0
SCALE = 100.0
NCORES = 8
B_SH = B // NCORES  # 2 samples per core
P = B_SH * T        # 128 partitions = (b, t)
K = 18              # (3 precision terms) x (3 dims) x (2 b-indicator)

_LIN = np.linspace(-RADIUS, RADIUS, R, dtype=np.float32)
BF16 = ml_dtypes.bfloat16


def build_bass(scratch_sbuf=True):
    nc = bacc.Bacc("TRN2", target_bir_lowering=False, name="ect_transform")
    x18 = nc.dram_tensor("x18", (K, N), mybir.dt.bfloat16, kind="ExternalInput")
    w18 = nc.dram_tensor("w18", (K, P), mybir.dt.bfloat16, kind="ExternalInput")
    bt = nc.dram_tensor("bt", (P, R), mybir.dt.float32, kind="ExternalInput")
    out = nc.dram_tensor("out", (B_SH, R, T), mybir.dt.float32, kind="ExternalOutput")

    with (
        tile.TileContext(nc) as tc,
        tc.tile_pool(name="sb", bufs=1) as sb,
        tc.tile_pool(name="sp", bufs=2) as sp,
        tc.tile_pool(name="ps", bufs=1, space="PSUM") as ps,
    ):
        # Warm the sigmoid activation table concurrently with input DMAs.
        warm = sb.tile([P, 1], mybir.dt.float32)
        nc.vector.memset(warm[:], 0.0)
        nc.scalar.activation(
            warm[:], warm[:], mybir.ActivationFunctionType.Sigmoid, bias=warm[:]
        )

        x18_sb = sb.tile([K, N], mybir.dt.bfloat16)
        w18_sb = sb.tile([K, P], mybir.dt.bfloat16)
        bt_sb = sb.tile([P, R], mybir.dt.float32)
        # x18 on the gpsimd SWDGE queue so it transfers concurrently with
        # w18/bt on the sync HWDGE queue (serial issue is ~0.7-1us each).
        nc.gpsimd.dma_start(out=x18_sb[:], in_=x18[:])
        nc.sync.dma_start(out=w18_sb[:], in_=w18[:])
        nc.sync.dma_start(out=bt_sb[:], in_=bt[:])

        ident = sb.tile([P, P], mybir.dt.float32)
        make_identity(nc, ident[:])
        ones = sb.tile([1, P], mybir.dt.float32)
        nc.vector.memset(ones[:], 1.0)

        nh_ps = ps.tile([P, N], mybir.dt.float32)  # 4 PSUM banks

        for j in range(N // 512):
            sl = slice(512 * j, 512 * (j + 1))
            nc.tensor.matmul(
                nh_ps[:, sl], w18_sb[:], x18_sb[:, sl], start=True, stop=True
            )

        # ACT writes bf16 sigmoid tiles (double-buffered); the otherwise-idle
        # DVE does the n-sums behind it via its own fused accumulator.  This
        # removes the per-instruction ACTIVATION_READ_ACCUMULATOR (~283ns)
        # from the ACT critical path.
        ect = sb.tile([P, R], mybir.dt.float32)
        for r in range(R):
            sg = sp.tile([P, N], mybir.dt.bfloat16)
            nc.scalar.activation(
                sg[:],
                nh_ps[:],
                mybir.ActivationFunctionType.Sigmoid,
                bias=bt_sb[:, r : r + 1],
                scale=-SCALE,
            )
            nc.vector.tensor_scalar(
                out=sg[:],
                in0=sg[:],
                scalar1=1.0,
                scalar2=None,
                op0=mybir.AluOpType.mult,
                op1=mybir.AluOpType.add,
                accum_out=ect[:, r : r + 1],
            )

        # Per-sample normalization: max over (t, r) within each b half.
        # All cross-partition movement happens on the PE (transpose +
        # K=1 broadcast matmul) -- no gpsimd, no DMA round-trips.
        m = sb.tile([P, 1], mybir.dt.float32)
        nc.vector.tensor_reduce(
            m[:], ect[:], axis=mybir.AxisListType.X, op=mybir.AluOpType.max
        )
        mT_ps = ps.tile([1, P], mybir.dt.float32)
        nc.tensor.transpose(mT_ps[:], m[:], ident[:])
        m2 = sb.tile([1, B_SH], mybir.dt.float32)
        nc.vector.tensor_reduce(
            m2[:],
            mT_ps.rearrange("p (b t) -> p b t", b=B_SH),
            axis=mybir.AxisListType.X,
            op=mybir.AluOpType.max,
        )
        rec2 = sb.tile([1, B_SH], mybir.dt.float32)
        nc.vector.reciprocal(rec2[:], m2[:])
        recb_ps = ps.tile([P, B_SH], mybir.dt.float32)
        nc.tensor.matmul(recb_ps[:], ones[:], rec2[:], start=True, stop=True)
        recb = sb.tile([P, B_SH], mybir.dt.float32)
        nc.vector.tensor_copy(recb[:], recb_ps[:])
        ectn = sb.tile([P, R], mybir.dt.float32)
        for b in range(B_SH):
            nc.vector.tensor_scalar_mul(
                ectn[b * T : (b + 1) * T, :],
                ect[b * T : (b + 1) * T, :],
                recb[b * T : (b + 1) * T, b : b + 1],
            )

        # Transpose (b,t) x r -> r x (b,t) on the PE, then contiguous DMAs.
        tp_ps = ps.tile([R, P], mybir.dt.float32)
        nc.tensor.transpose(tp_ps[:], ectn[:], ident[:])
        out_sb = sb.tile([R, P], mybir.dt.float32)
        nc.vector.tensor_copy(out_sb[:], tp_ps[:])
        for b in range(B_SH):
            nc.sync.dma_start(
                out=out[b], in_=out_sb[:, b * T : (b + 1) * T]
            )

    nc.compile()
    return nc


def _make_w18_x18(v, xs):
    """xs: (2, N, D) f32 shard.  Returns (w18 (K,P) bf16, x18 (K,N) bf16).

    Row k = s*6 + d*2 + kb encodes precision term s, dim d, sample kb:
      s=0: x_hi * v_hi ; s=1: x_lo * v_hi ; s=2: x_hi * v_lo
    """
    v_hi = v.astype(BF16)
    v_lo = (v - v_hi.astype(np.float32)).astype(BF16)
    x_hi = xs.astype(BF16)
    x_lo = (xs - x_hi.astype(np.float32)).astype(BF16)
    w18 = np.zeros((K, P), dtype=BF16)
    x18 = np.zeros((K, N), dtype=BF16)
    for s, (vv, xx) in enumerate(((v_hi, x_hi), (v_hi, x_lo), (v_lo, x_hi))):
        for d in range(D):
            for kb in range(B_SH):
                w18[s * 6 + d * 2 + kb, kb * T : (kb + 1) * T] = vv[d]
                x18[s * 6 + d * 2 + kb, :] = xx[kb, :, d]
    return w18, x18


def _make_bt():
    # bias table: column r = SCALE*lin[r], replicated across partitions
    return np.ascontiguousarray(
        np.tile((SCALE * _LIN)[None, :], (P, 1)).astype(np.float32)
    )


_NC_CACHE = {}


def _get_nc():
    if "nc" not in _NC_CACHE:
        _NC_CACHE["nc"] = build_bass()
    return _NC_CACHE["nc"]


def kernel(x, v, _trace=False, _nc=None):
    x = np.ascontiguousarray(np.asarray(x, dtype=np.float32))
    v = np.ascontiguousarray(np.asarray(v, dtype=np.float32))
    assert x.shape == (B, N, D) and v.shape == (D, T)

    bt = _make_bt()
    in_maps = []
    for c in range(NCORES):
        w18, x18 = _make_w18_x18(v, x[B_SH * c : B_SH * (c + 1)])
        in_maps.append({"x18": x18, "w18": w18, "bt": bt})

    nc = _nc if _nc is not None else _get_nc()
    res = run_bass_kernel_spmd(
        nc, in_maps, core_ids=list(range(NCORES)), trace=_trace
    )
    out = np.concatenate([r["out"] for r in res.results], axis=0)
    if _trace:
        return out.astype(np.float32), res
    return out.astype(np.float32)


# revision 26
# speedup vs baseline: 1.0183x; 1.0183x over previous
"""ECT transform kernel for Trainium2, SPMD over 8 NeuronCores.

Math (per sample b):
    nh[b,n,t] = sum_d x[b,n,d] * v[d,t]
    ect[b,r,t] = sum_n sigmoid(SCALE*(lin[r] - nh[b,n,t]))
    out[b] = ect[b] / max_{r,t} ect[b]

Sharding: data-parallel over batch (B=16 -> 2 samples per core).

Per-core program (partitions p = (b, t), b in {0,1}, t in 0..63):
  - PE: nh[(b,t), n] = w18.T @ x18, K=18 = (split, d, b-indicator) with
    bf16-split precision (x_hi*v_hi + x_lo*v_hi + x_hi*v_lo), 4 matmuls of
    N=512 into one PSUM tile (128, 2048).  Single-pass bf16 (fp32 matmul
    runs as two passes on this PE).
  - ACT: for each r: sigmoid(-SCALE*nh + SCALE*lin[r]) over the whole PSUM
    tile, with the fused per-partition accumulator (accum_out) producing
    ect[(b,t), r] directly.  The r axis is never materialized.
  - normalize: free-dim max (DVE) + per-half partition max (GPSIMD at
    base partition 0), reciprocal, per-partition scale.
  - PE-transpose (identity matmul) to (r, (b,t)) and two contiguous
    output DMAs (strided 4-byte DMA to DRAM measured ~24us; transpose
    path is ~1us).
"""

import numpy as np
import ml_dtypes

import concourse.bacc as bacc
import concourse.tile as tile
from concourse import mybir
from concourse.bass_utils import run_bass_kernel_spmd
from concourse.masks import make_identity

B = 16
N = 2048
D = 3
T = 64
R = 64
RADIUS = 1.0
SCALE = 100.0
NCORES = 8
B_SH = B // NCORES  # 2 samples per core
P = B_SH * T        # 128 partitions = (b, t)
K = 18              # (3 precision terms) x (3 dims) x (2 b-indicator)

_LIN = np.linspace(-RADIUS, RADIUS, R, dtype=np.float32)
BF16 = ml_dtypes.bfloat16


def build_bass(scratch_sbuf=True):
    nc = bacc.Bacc("TRN2", target_bir_lowering=False, name="ect_transform")
    x18 = nc.dram_tensor("x18", (K, N), mybir.dt.bfloat16, kind="ExternalInput")
    w18 = nc.dram_tensor("w18", (K, P), mybir.dt.bfloat16, kind="ExternalInput")
    bt = nc.dram_tensor("bt", (P, R), mybir.dt.float32, kind="ExternalInput")
    out = nc.dram_tensor("out", (B_SH, R, T), mybir.dt.float32, kind="ExternalOutput")

    with (
        tile.TileContext(nc) as tc,
        tc.tile_pool(name="sb", bufs=1) as sb,
        tc.tile_pool(name="sp", bufs=2) as sp,
        tc.tile_pool(name="ps", bufs=1, space="PSUM") as ps,
    ):
        # Warm the sigmoid activation table concurrently with input DMAs.
        warm = sb.tile([P, 1], mybir.dt.float32)
        nc.vector.memset(warm[:], 0.0)
        nc.scalar.activation(
            warm[:], warm[:], mybir.ActivationFunctionType.Sigmoid, bias=warm[:]
        )

        x18_sb = sb.tile([K, N], mybir.dt.bfloat16)
        w18_sb = sb.tile([K, P], mybir.dt.bfloat16)
        bt_sb = sb.tile([P, R], mybir.dt.float32)
        # x18 on the gpsimd SWDGE queue so it transfers concurrently with
        # w18/bt on the sync HWDGE queue (serial issue is ~0.7-1us each).
        nc.gpsimd.dma_start(out=x18_sb[:], in_=x18[:])
        nc.sync.dma_start(out=w18_sb[:], in_=w18[:])
        nc.sync.dma_start(out=bt_sb[:], in_=bt[:])

        ident = sb.tile([P, P], mybir.dt.float32)
        make_identity(nc, ident[:])
        ones = sb.tile([1, P], mybir.dt.float32)
        nc.vector.memset(ones[:], 1.0)

        nh_ps = ps.tile([P, N], mybir.dt.float32)  # 4 PSUM banks

        for j in range(N // 512):
            sl = slice(512 * j, 512 * (j + 1))
            nc.tensor.matmul(
                nh_ps[:, sl], w18_sb[:], x18_sb[:, sl], start=True, stop=True
            )

        # Hybrid n-sum: most thresholds use a plain ACTIVATE (1967ns) with
        # the n-sum done behind it on the otherwise-idle DVE (2280ns fused
        # tensor_scalar reduce, hidden under ACT); every 5th threshold keeps
        # ACT's own accumulator (2182ns incl READ_ACC) so the DVE backlog
        # never exceeds ACT's pace.  Balances both engines.
        ect = sb.tile([P, R], mybir.dt.float32)
        scr = sb.tile([P, N], mybir.dt.bfloat16)
        for r in range(R):
            if r % 5 == 0:
                nc.scalar.activation(
                    scr[:],
                    nh_ps[:],
                    mybir.ActivationFunctionType.Sigmoid,
                    bias=bt_sb[:, r : r + 1],
                    scale=-SCALE,
                    accum_out=ect[:, r : r + 1],
                )
            else:
                sg = sp.tile([P, N], mybir.dt.bfloat16)
                nc.scalar.activation(
                    sg[:],
                    nh_ps[:],
                    mybir.ActivationFunctionType.Sigmoid,
                    bias=bt_sb[:, r : r + 1],
                    scale=-SCALE,
                )
                nc.vector.tensor_scalar(
                    out=sg[:],
                    in0=sg[:],
                    scalar1=1.0,
                    scalar2=None,
                    op0=mybir.AluOpType.mult,
                    op1=mybir.AluOpType.add,
                    accum_out=ect[:, r : r + 1],
                )

        # Per-sample normalization: max over (t, r) within each b half.
        # All cross-partition movement happens on the PE (transpose +
        # K=1 broadcast matmul) -- no gpsimd, no DMA round-trips.
        m = sb.tile([P, 1], mybir.dt.float32)
        nc.vector.tensor_reduce(
            m[:], ect[:], axis=mybir.AxisListType.X, op=mybir.AluOpType.max
        )
        mT_ps = ps.tile([1, P], mybir.dt.float32)
        nc.tensor.transpose(mT_ps[:], m[:], ident[:])
        m2 = sb.tile([1, B_SH], mybir.dt.float32)
        nc.vector.tensor_reduce(
            m2[:],
            mT_ps.rearrange("p (b t) -> p b t", b=B_SH),
            axis=mybir.AxisListType.X,
            op=mybir.AluOpType.max,
        )
        rec2 = sb.tile([1, B_SH], mybir.dt.float32)
        nc.vector.reciprocal(rec2[:], m2[:])
        recb_ps = ps.tile([P, B_SH], mybir.dt.float32)
        nc.tensor.matmul(recb_ps[:], ones[:], rec2[:], start=True, stop=True)
        recb = sb.tile([P, B_SH], mybir.dt.float32)
        nc.vector.tensor_copy(recb[:], recb_ps[:])
        ectn = sb.tile([P, R], mybir.dt.float32)
        for b in range(B_SH):
            nc.vector.tensor_scalar_mul(
                ectn[b * T : (b + 1) * T, :],
                ect[b * T : (b + 1) * T, :],
                recb[b * T : (b + 1) * T, b : b + 1],
            )

        # Transpose (b,t) x r -> r x (b,t) on the PE, then contiguous DMAs.
        tp_ps = ps.tile([R, P], mybir.dt.float32)
        nc.tensor.transpose(tp_ps[:], ectn[:], ident[:])
        out_sb = sb.tile([R, P], mybir.dt.float32)
        nc.vector.tensor_copy(out_sb[:], tp_ps[:])
        for b in range(B_SH):
            nc.sync.dma_start(
                out=out[b], in_=out_sb[:, b * T : (b + 1) * T]
            )

    nc.compile()
    return nc


def _make_w18_x18(v, xs):
    """xs: (2, N, D) f32 shard.  Returns (w18 (K,P) bf16, x18 (K,N) bf16).

    Row k = s*6 + d*2 + kb encodes precision term s, dim d, sample kb:
      s=0: x_hi * v_hi ; s=1: x_lo * v_hi ; s=2: x_hi * v_lo
    """
    v_hi = v.astype(BF16)
    v_lo = (v - v_hi.astype(np.float32)).astype(BF16)
    x_hi = xs.astype(BF16)
    x_lo = (xs - x_hi.astype(np.float32)).astype(BF16)
    w18 = np.zeros((K, P), dtype=BF16)
    x18 = np.zeros((K, N), dtype=BF16)
    for s, (vv, xx) in enumerate(((v_hi, x_hi), (v_hi, x_lo), (v_lo, x_hi))):
        for d in range(D):
            for kb in range(B_SH):
                w18[s * 6 + d * 2 + kb, kb * T : (kb + 1) * T] = vv[d]
                x18[s * 6 + d * 2 + kb, :] = xx[kb, :, d]
    return w18, x18


def _make_bt():
    # bias table: column r = SCALE*lin[r], replicated across partitions
    return np.ascontiguousarray(
        np.tile((SCALE * _LIN)[None, :], (P, 1)).astype(np.float32)
    )


_NC_CACHE = {}


def _get_nc():
    if "nc" not in _NC_CACHE:
        _NC_CACHE["nc"] = build_bass()
    return _NC_CACHE["nc"]


def kernel(x, v, _trace=False, _nc=None):
    x = np.ascontiguousarray(np.asarray(x, dtype=np.float32))
    v = np.ascontiguousarray(np.asarray(v, dtype=np.float32))
    assert x.shape == (B, N, D) and v.shape == (D, T)

    bt = _make_bt()
    in_maps = []
    for c in range(NCORES):
        w18, x18 = _make_w18_x18(v, x[B_SH * c : B_SH * (c + 1)])
        in_maps.append({"x18": x18, "w18": w18, "bt": bt})

    nc = _nc if _nc is not None else _get_nc()
    res = run_bass_kernel_spmd(
        nc, in_maps, core_ids=list(range(NCORES)), trace=_trace
    )
    out = np.concatenate([r["out"] for r in res.results], axis=0)
    if _trace:
        return out.astype(np.float32), res
    return out.astype(np.float32)


# revision 27
# speedup vs baseline: 1.0666x; 1.0475x over previous
"""ECT transform kernel for Trainium2, SPMD over 8 NeuronCores.

Math (per sample b):
    nh[b,n,t] = sum_d x[b,n,d] * v[d,t]
    ect[b,r,t] = sum_n sigmoid(SCALE*(lin[r] - nh[b,n,t]))
    out[b] = ect[b] / max_{r,t} ect[b]

Sharding: data-parallel over batch (B=16 -> 2 samples per core).

Per-core program (partitions p = (b, t), b in {0,1}, t in 0..63):
  - PE: nh[(b,t), n] = w18.T @ x18, K=18 = (split, d, b-indicator) with
    bf16-split precision (x_hi*v_hi + x_lo*v_hi + x_hi*v_lo), 4 matmuls of
    N=512 into one PSUM tile (128, 2048).  Single-pass bf16 (fp32 matmul
    runs as two passes on this PE).
  - ACT: for each r: sigmoid(-SCALE*nh + SCALE*lin[r]) over the whole PSUM
    tile, with the fused per-partition accumulator (accum_out) producing
    ect[(b,t), r] directly.  The r axis is never materialized.
  - normalize: free-dim max (DVE) + per-half partition max (GPSIMD at
    base partition 0), reciprocal, per-partition scale.
  - PE-transpose (identity matmul) to (r, (b,t)) and two contiguous
    output DMAs (strided 4-byte DMA to DRAM measured ~24us; transpose
    path is ~1us).
"""

import numpy as np
import ml_dtypes

import concourse.bacc as bacc
import concourse.tile as tile
from concourse import mybir
from concourse.bass_utils import run_bass_kernel_spmd
from concourse.masks import make_identity

B = 16
N = 2048
D = 3
T = 64
R = 64
RADIUS = 1.0
SCALE = 100.0
NCORES = 8
B_SH = B // NCORES  # 2 samples per core
P = B_SH * T        # 128 partitions = (b, t)
K = 18              # (3 precision terms) x (3 dims) x (2 b-indicator)

_LIN = np.linspace(-RADIUS, RADIUS, R, dtype=np.float32)
BF16 = ml_dtypes.bfloat16


def build_bass(scratch_sbuf=True):
    nc = bacc.Bacc("TRN2", target_bir_lowering=False, name="ect_transform")
    x18 = nc.dram_tensor("x18", (K, N), mybir.dt.bfloat16, kind="ExternalInput")
    w18 = nc.dram_tensor("w18", (K, P), mybir.dt.bfloat16, kind="ExternalInput")
    bt = nc.dram_tensor("bt", (P, R), mybir.dt.float32, kind="ExternalInput")
    out = nc.dram_tensor("out", (B_SH, R, T), mybir.dt.float32, kind="ExternalOutput")

    with (
        tile.TileContext(nc) as tc,
        tc.tile_pool(name="sb", bufs=1) as sb,
        tc.tile_pool(name="sp", bufs=6) as sp,
        tc.tile_pool(name="ps", bufs=1, space="PSUM") as ps,
    ):
        # Warm the sigmoid activation table concurrently with input DMAs.
        warm = sb.tile([P, 1], mybir.dt.float32)
        nc.vector.memset(warm[:], 0.0)
        nc.scalar.activation(
            warm[:], warm[:], mybir.ActivationFunctionType.Sigmoid, bias=warm[:]
        )

        x18_sb = sb.tile([K, N], mybir.dt.bfloat16)
        w18_sb = sb.tile([K, P], mybir.dt.bfloat16)
        bt_sb = sb.tile([P, R], mybir.dt.float32)
        # x18 on the gpsimd SWDGE queue so it transfers concurrently with
        # w18/bt on the sync HWDGE queue (serial issue is ~0.7-1us each).
        nc.gpsimd.dma_start(out=x18_sb[:], in_=x18[:])
        nc.sync.dma_start(out=w18_sb[:], in_=w18[:])
        nc.sync.dma_start(out=bt_sb[:], in_=bt[:])

        ident = sb.tile([P, P], mybir.dt.float32)
        make_identity(nc, ident[:])
        ones = sb.tile([1, P], mybir.dt.float32)
        nc.vector.memset(ones[:], 1.0)

        nh_ps = ps.tile([P, N], mybir.dt.float32)  # 4 PSUM banks

        for j in range(N // 512):
            sl = slice(512 * j, 512 * (j + 1))
            nc.tensor.matmul(
                nh_ps[:, sl], w18_sb[:], x18_sb[:, sl], start=True, stop=True
            )

        # Hybrid n-sum: most thresholds use a plain ACTIVATE (1967ns) with
        # the n-sum done behind it on the otherwise-idle DVE (2280ns fused
        # tensor_scalar reduce, hidden under ACT); every 5th threshold keeps
        # ACT's own accumulator (2182ns incl READ_ACC) so the DVE backlog
        # never exceeds ACT's pace.  Balances both engines.
        ect = sb.tile([P, R], mybir.dt.float32)
        scr = sb.tile([P, N], mybir.dt.bfloat16)
        for r in range(R):
            if r % 5 == 0:
                nc.scalar.activation(
                    scr[:],
                    nh_ps[:],
                    mybir.ActivationFunctionType.Sigmoid,
                    bias=bt_sb[:, r : r + 1],
                    scale=-SCALE,
                    accum_out=ect[:, r : r + 1],
                )
            else:
                sg = sp.tile([P, N], mybir.dt.bfloat16)
                nc.scalar.activation(
                    sg[:],
                    nh_ps[:],
                    mybir.ActivationFunctionType.Sigmoid,
                    bias=bt_sb[:, r : r + 1],
                    scale=-SCALE,
                )
                nc.vector.tensor_scalar(
                    out=sg[:],
                    in0=sg[:],
                    scalar1=1.0,
                    scalar2=None,
                    op0=mybir.AluOpType.mult,
                    op1=mybir.AluOpType.add,
                    accum_out=ect[:, r : r + 1],
                )

        # Per-sample normalization: max over (t, r) within each b half.
        # All cross-partition movement happens on the PE (transpose +
        # K=1 broadcast matmul) -- no gpsimd, no DMA round-trips.
        m = sb.tile([P, 1], mybir.dt.float32)
        nc.vector.tensor_reduce(
            m[:], ect[:], axis=mybir.AxisListType.X, op=mybir.AluOpType.max
        )
        mT_ps = ps.tile([1, P], mybir.dt.float32)
        nc.tensor.transpose(mT_ps[:], m[:], ident[:])
        m2 = sb.tile([1, B_SH], mybir.dt.float32)
        nc.vector.tensor_reduce(
            m2[:],
            mT_ps.rearrange("p (b t) -> p b t", b=B_SH),
            axis=mybir.AxisListType.X,
            op=mybir.AluOpType.max,
        )
        rec2 = sb.tile([1, B_SH], mybir.dt.float32)
        nc.vector.reciprocal(rec2[:], m2[:])
        recb_ps = ps.tile([P, B_SH], mybir.dt.float32)
        nc.tensor.matmul(recb_ps[:], ones[:], rec2[:], start=True, stop=True)
        recb = sb.tile([P, B_SH], mybir.dt.float32)
        nc.vector.tensor_copy(recb[:], recb_ps[:])
        ectn = sb.tile([P, R], mybir.dt.float32)
        for b in range(B_SH):
            nc.vector.tensor_scalar_mul(
                ectn[b * T : (b + 1) * T, :],
                ect[b * T : (b + 1) * T, :],
                recb[b * T : (b + 1) * T, b : b + 1],
            )

        # Transpose (b,t) x r -> r x (b,t) on the PE, then contiguous DMAs.
        tp_ps = ps.tile([R, P], mybir.dt.float32)
        nc.tensor.transpose(tp_ps[:], ectn[:], ident[:])
        out_sb = sb.tile([R, P], mybir.dt.float32)
        nc.vector.tensor_copy(out_sb[:], tp_ps[:])
        for b in range(B_SH):
            nc.sync.dma_start(
                out=out[b], in_=out_sb[:, b * T : (b + 1) * T]
            )

    nc.compile()
    return nc


def _make_w18_x18(v, xs):
    """xs: (2, N, D) f32 shard.  Returns (w18 (K,P) bf16, x18 (K,N) bf16).

    Row k = s*6 + d*2 + kb encodes precision term s, dim d, sample kb:
      s=0: x_hi * v_hi ; s=1: x_lo * v_hi ; s=2: x_hi * v_lo
    """
    v_hi = v.astype(BF16)
    v_lo = (v - v_hi.astype(np.float32)).astype(BF16)
    x_hi = xs.astype(BF16)
    x_lo = (xs - x_hi.astype(np.float32)).astype(BF16)
    w18 = np.zeros((K, P), dtype=BF16)
    x18 = np.zeros((K, N), dtype=BF16)
    for s, (vv, xx) in enumerate(((v_hi, x_hi), (v_hi, x_lo), (v_lo, x_hi))):
        for d in range(D):
            for kb in range(B_SH):
                w18[s * 6 + d * 2 + kb, kb * T : (kb + 1) * T] = vv[d]
                x18[s * 6 + d * 2 + kb, :] = xx[kb, :, d]
    return w18, x18


def _make_bt():
    # bias table: column r = SCALE*lin[r], replicated across partitions
    return np.ascontiguousarray(
        np.tile((SCALE * _LIN)[None, :], (P, 1)).astype(np.float32)
    )


_NC_CACHE = {}


def _get_nc():
    if "nc" not in _NC_CACHE:
        _NC_CACHE["nc"] = build_bass()
    return _NC_CACHE["nc"]


def kernel(x, v, _trace=False, _nc=None):
    x = np.ascontiguousarray(np.asarray(x, dtype=np.float32))
    v = np.ascontiguousarray(np.asarray(v, dtype=np.float32))
    assert x.shape == (B, N, D) and v.shape == (D, T)

    bt = _make_bt()
    in_maps = []
    for c in range(NCORES):
        w18, x18 = _make_w18_x18(v, x[B_SH * c : B_SH * (c + 1)])
        in_maps.append({"x18": x18, "w18": w18, "bt": bt})

    nc = _nc if _nc is not None else _get_nc()
    res = run_bass_kernel_spmd(
        nc, in_maps, core_ids=list(range(NCORES)), trace=_trace
    )
    out = np.concatenate([r["out"] for r in res.results], axis=0)
    if _trace:
        return out.astype(np.float32), res
    return out.astype(np.float32)
